# revision 24
# baseline (speedup 1.0000x reference)
"""ChromaSelfAttention on 8 TRN2 NeuronCores (Bass/Tile, SPMD).

Problem (hardcoded): B=2, L=2048, D=2048, H=16 heads, head_dim=128.
    q = x_q @ Wq + bq ; k = x_k @ Wk + bk ; v = x_v @ Wv + bv   (per batch)
    o = softmax(q k^T / sqrt(128)) v                            (per b,h)
    y = o @ Wo + bo
Sharding: core c handles batch b=c//4 and the 4 heads starting at
(c%4)*4 (data + head parallel). Each core computes a partial y for its
batch from its 4 heads; a ReduceScatter over the 4-core batch group
sums partials (bf16). bo is added on the host during assembly.

Schedule (single PE queue, emission order = execution order):
  V-proj (all 16 j-chunks) -> K-proj heads {0,1} -> Q-proj block 0 ->
  S(0,0) -> K-proj heads {2,3} -> attention steady state with Q-proj
  of block n+1 and the out-projection+RS of block n interleaved into
  the PE stream. Within a step, S(u+1) and O(u) matmuls interleave in
  half-groups so exp(u) latency hides and pts tiles stay at 12 bufs.
  First RS chunk is 128 rows so the (BW-bound) collective stream
  starts as early as possible; subsequent chunks are larger; the tail
  chunks are small again.

Everything is bf16 on the PE (x/W cast on host - measured same PE rate
as f32r, half the DMA bytes); psums f32; softmax colsum via DVE
pair-tree + GPSIMD partition all-reduce (gpsimd is otherwise idle).
"""
import ml_dtypes
import numpy as np

import concourse.bacc as bacc
import concourse.bass_isa as bass_isa
import concourse.tile as tile
import concourse.mybir as mybir

F32 = mybir.dt.float32
BF16 = mybir.dt.bfloat16
AF = mybir.ActivationFunctionType
AOP = mybir.AluOpType

B = 2
L = 2048
D = 2048
HD = 128
HLOC = 4              # heads per core
HDL = HLOC * HD       # 512 local hd columns
NK = D // 128         # 16 contraction chunks
NI = L // 512         # 4 i-blocks
NI128 = L // 128      # 16 i/j 128-chunks
SCALE = HD ** -0.5
GROUPS = [[0, 1, 2, 3], [4, 5, 6, 7]]

# RS chunk table: tiny first chunk (start the stream ASAP), big middle,
# small tail. (absolute-row-offset, nrows). Block 3 is processed as two
# 256-row sub-blocks so its RS mostly hides under remaining compute.
CHUNKS = [(0, 128), (128, 384), (512, 512), (1024, 512),
          (1536, 256), (1792, 128), (1920, 128)]

_CACHE = {}


def _build():
    nc = bacc.Bacc("TRN2", target_bir_lowering=False, debug=False,
                   num_devices=8)
    xqt = nc.dram_tensor("xqt", [D, L], BF16, kind="ExternalInput").ap()
    xkt = nc.dram_tensor("xkt", [D, L], BF16, kind="ExternalInput").ap()
    xvt = nc.dram_tensor("xvt", [D, L], BF16, kind="ExternalInput").ap()
    wq = nc.dram_tensor("wq", [D, HDL], BF16, kind="ExternalInput").ap()
    wk = nc.dram_tensor("wk", [D, HDL], BF16, kind="ExternalInput").ap()
    wv = nc.dram_tensor("wv", [D, HDL], BF16, kind="ExternalInput").ap()
    wo = nc.dram_tensor("wo", [HDL, D], BF16, kind="ExternalInput").ap()
    bq2 = nc.dram_tensor("bq2", [HLOC, 128, 1], F32, kind="ExternalInput").ap()
    bk2 = nc.dram_tensor("bk2", [HLOC, 128, 1], F32, kind="ExternalInput").ap()
    bv2 = nc.dram_tensor("bv2", [1, HDL], F32, kind="ExternalInput").ap()
    y = nc.dram_tensor("y", [512, D], BF16, kind="ExternalOutput").ap()

    ypart = [nc.dram_tensor(f"ypart{q}", [r, D], BF16)
             for q, (_, r) in enumerate(CHUNKS)]
    yred = [nc.dram_tensor(f"yred{q}", [r // 4, D], BF16)
            for q, (_, r) in enumerate(CHUNKS)]

    with tile.TileContext(nc) as tc:
        with tc.tile_pool(name="const", bufs=1) as cp, \
             tc.tile_pool(name="ps", bufs=1, space="PSUM") as psp, \
             tc.tile_pool(name="wA", bufs=1) as wap, \
             tc.tile_pool(name="wk", bufs=1) as wkp, \
             tc.tile_pool(name="wq", bufs=1) as wqp, \
             tc.tile_pool(name="xt", bufs=5) as xtp, \
             tc.tile_pool(name="qkv", bufs=1) as qkvp, \
             tc.tile_pool(name="ptp", bufs=12) as ptp, \
             tc.tile_pool(name="accp", bufs=5) as accp, \
             tc.tile_pool(name="csp", bufs=2) as csp, \
             tc.tile_pool(name="rbp", bufs=2) as rbp, \
             tc.tile_pool(name="otp", bufs=1) as otp, \
             tc.tile_pool(name="ysp", bufs=3) as ysp:
            # ---- constants / biases ----
            bq_t = []
            bk_t = []
            for m in range(HLOC):
                t = cp.tile([128, 1], F32, name=f"bq_{m}", tag="bq",
                            bufs=HLOC)
                nc.gpsimd.dma_start(t, bq2[m])
                bq_t.append(t)
                t = cp.tile([128, 1], F32, name=f"bk_{m}", tag="bk",
                            bufs=HLOC)
                nc.gpsimd.dma_start(t, bk2[m])
                bk_t.append(t)
            bv_t = cp.tile([1, HDL], F32, name="bv_t")
            nc.gpsimd.dma_start(bv_t, bv2)
            bv_b = cp.tile([128, HDL], F32, name="bv_b")
            nc.gpsimd.partition_broadcast(bv_b, bv_t)

            # ---- persistent sbuf tensors ----
            qt = [qkvp.tile([128, L], BF16, name=f"qt{m}", tag="qt",
                            bufs=HLOC) for m in range(HLOC)]
            kt = [qkvp.tile([128, L], BF16, name=f"kt{m}", tag="kt",
                            bufs=HLOC) for m in range(HLOC)]
            vv = [qkvp.tile([128, HDL], BF16, name=f"vv{c}", tag="vv",
                            bufs=NI128) for c in range(NI128)]
            ot = [otp.tile([128, L], BF16, name=f"ot{h}", tag="ot",
                           bufs=HLOC) for h in range(HLOC)]

            def load_w(pool, wd, nm, tag):
                ts = []
                for k in range(NK):
                    t = pool.tile([128, HDL], BF16, name=f"{nm}{k}",
                                  tag=tag, bufs=NK)
                    nc.sync.dma_start(t, wd[k*128:(k+1)*128, :])
                    ts.append(t)
                return ts

            def load_xt(xd, nm, n, parts=1):
                """Two half tiles [128, 8*512] per (tensor, i-block);
                half hf covers k-chunks hf*8..hf*8+7; free index =
                (k%8)*512 + i. DMA'd via 3D AP in `parts` pieces so
                early matmuls can start before it all lands."""
                ts = []
                src3 = xd.rearrange("(k p) l -> p k l", p=128)[
                    :, :, n*512:(n+1)*512]
                for hf in range(2):
                    t = xtp.tile([128, 8 * 512], BF16,
                                 name=f"{nm}{n}_{hf}", tag="xt", bufs=5)
                    dst3 = t.rearrange("p (k i) -> p k i", k=8)
                    np_ = 8 if (parts > 1 and hf == 0) else 1
                    kq = 8 // np_
                    for pi in range(np_):
                        nc.sync.dma_start(
                            dst3[:, pi*kq:(pi+1)*kq, :],
                            src3[:, hf*8 + pi*kq:hf*8 + (pi+1)*kq, :])
                    ts.append(t)
                return ts

            rs_insts = []

            # ---------------- V projection (all chunks) ----------------
            wv_t = load_w(wap, wv, "wv", "wA")
            for n in range(NI):
                x_t = load_xt(xvt, "xv", n, parts=8 if n == 0 else 1)
                for mi in range(4):
                    ci = n * 4 + mi
                    ps = psp.tile([128, HDL], F32, tag="psA", bufs=2,
                                  name=f"psv{ci}")
                    for k in range(NK):
                        kk = (k % 8) * 512 + mi * 128
                        nc.tensor.matmul(
                            ps, x_t[k // 8][:, kk:kk+128],
                            wv_t[k],
                            start=(k == 0), stop=(k == NK - 1))
                    nc.vector.tensor_add(vv[ci], ps, bv_b)

            # ---------------- K projection, head-pass style -------------
            wk_t = load_w(wkp, wk, "wk", "wk")

            def kproj_pass(ms):
                for n in range(NI):
                    x_t = load_xt(xkt, f"xk{ms[0]}", n)
                    for m in ms:
                        ps = psp.tile([128, 512], F32, tag="psA",
                                      bufs=2, name=f"psk{n}{m}")
                        for k in range(NK):
                            kk = (k % 8) * 512
                            nc.tensor.matmul(
                                ps, wk_t[k][:, m*128:(m+1)*128],
                                x_t[k // 8][:, kk:kk+512],
                                start=(k == 0), stop=(k == NK - 1))
                        nc.scalar.activation(
                            kt[m][:, n*512:(n+1)*512], ps,
                            AF.Identity, bias=bk_t[m], scale=1.0)

            kproj_pass([0, 1])

            wq_t = load_w(wqp, wq, "wq", "wq")

            def qproj_block(n):
                x_t = load_xt(xqt, "xq", n)
                for m in range(HLOC):
                    ps = psp.tile([128, 512], F32, tag="psA",
                                  bufs=2, name=f"psq{n}{m}")
                    for k in range(NK):
                        kk = (k % 8) * 512
                        nc.tensor.matmul(
                            ps, wq_t[k][:, m*128:(m+1)*128],
                            x_t[k // 8][:, kk:kk+512],
                            start=(k == 0), stop=(k == NK - 1))
                    nc.scalar.activation(
                        qt[m][:, n*512:(n+1)*512], ps,
                        AF.Identity, bias=bq_t[m], scale=1.0)

            qproj_block(0)

            def qproj_head(n, m, x_t):
                ps = psp.tile([128, 512], F32, tag="psA",
                              bufs=2, name=f"psq{n}{m}")
                for k in range(NK):
                    kk = (k % 8) * 512
                    nc.tensor.matmul(
                        ps, wq_t[k][:, m*128:(m+1)*128],
                        x_t[k // 8][:, kk:kk+512],
                        start=(k == 0), stop=(k == NK - 1))
                nc.scalar.activation(
                    qt[m][:, n*512:(n+1)*512], ps,
                    AF.Identity, bias=bq_t[m], scale=1.0)

            # ---------------- attention machinery -----------------------
            # A "step" is (i0, w, h): rows [i0, i0+w) of head h. Each
            # pts/psS tile is [128, 1024] holding g = 1024//w j-chunks;
            # a step has T = 16//g such tiles.
            def attn_S(i0, w, h, trng):
                """S^T matmuls + exp for pts-tile indices trng."""
                g = 1024 // w
                pts = []
                for t in trng:
                    sp = psp.tile([128, 1024], F32, tag="psS",
                                  bufs=2, name=f"sp{i0}_{h}_{t}")
                    for qq in range(g):
                        c = t * g + qq
                        nc.tensor.matmul(
                            sp[:, qq*w:(qq+1)*w],
                            kt[h][:, c*128:(c+1)*128],
                            qt[h][:, i0:i0+w],
                            start=True, stop=True)
                    p = ptp.tile([128, 1024], BF16, tag="pt",
                                 name=f"p{i0}_{h}_{t}")
                    nc.scalar.activation(p, sp, AF.Exp, scale=SCALE)
                    pts.append(p)
                return pts

            def attn_tree(i0, w, h, pts):
                """DVE colsum tree -> cs [128, w] f32 partials
                (per j-residue partition; summed across partitions by
                the gpsimd all-reduce later)."""
                nm = f"{i0}_{h}"
                if w == 512:
                    a2 = []
                    for j in range(4):
                        t = accp.tile([128, 1024], BF16, tag="acc",
                                      name=f"acc{nm}_{j}")
                        nc.vector.tensor_add(t, pts[2*j], pts[2*j+1])
                        a2.append(t)
                    nc.vector.tensor_add(a2[0], a2[0], a2[1])
                    nc.vector.tensor_add(a2[2], a2[2], a2[3])
                    nc.vector.tensor_add(a2[0], a2[0], a2[2])
                    cs = csp.tile([128, 512], F32, tag="cs",
                                  name=f"cs{nm}")
                    nc.vector.tensor_add(cs, a2[0][:, 0:512],
                                         a2[0][:, 512:1024])
                    return cs
                # w == 256: 4 pts tiles of 4 chunk-groups each
                b0 = accp.tile([128, 1024], BF16, tag="acc",
                               name=f"accb0{nm}")
                nc.vector.tensor_add(b0, pts[0], pts[1])
                b1 = accp.tile([128, 1024], BF16, tag="acc",
                               name=f"accb1{nm}")
                nc.vector.tensor_add(b1, pts[2], pts[3])
                g2 = accp.tile([128, 1024], BF16, tag="acc",
                               name=f"accg{nm}")
                nc.vector.tensor_add(g2[:, 0:512], b0[:, 0:512],
                                     b0[:, 512:1024])
                nc.vector.tensor_add(g2[:, 512:1024], b1[:, 0:512],
                                     b1[:, 512:1024])
                f2 = accp.tile([128, 1024], BF16, tag="acc",
                               name=f"accf{nm}")
                nc.vector.tensor_add(f2[:, 0:512], g2[:, 0:512],
                                     g2[:, 512:1024])
                cs = csp.tile([128, 512], F32, tag="cs", name=f"cs{nm}")
                nc.vector.tensor_add(cs[:, 0:256], f2[:, 0:256],
                                     f2[:, 256:512])
                return cs

            def attn_O(i0, w, h, pts, op, cs, rng):
                """O^T matmul chunk-range rng; on the last chunk also
                run the normalize chain."""
                g = 1024 // w
                ops = op[:, 0:w]
                for c in rng:
                    nc.tensor.matmul(
                        ops, vv[c][:, h*128:(h+1)*128],
                        pts[c // g][:, (c % g)*w:(c % g)*w+w],
                        start=(c == 0), stop=(c == NI128 - 1))
                if rng[-1] == NI128 - 1:
                    nc.vector.tensor_copy(ot[h][:, i0:i0+w], ops)
                    nc.gpsimd.partition_all_reduce(
                        cs[:, 0:w], cs[:, 0:w], 128,
                        bass_isa.ReduceOp.add)
                    rb = rbp.tile([128, 512], F32, tag="rb",
                                  name=f"rb{i0}_{h}")
                    nc.vector.reciprocal_approx_fast(rb[:, 0:w],
                                                     cs[:, 0:w])
                    sl = ot[h][:, i0:i0+w]
                    nc.vector.tensor_mul(sl, sl, rb[:, 0:w])

            wo_t = None

            def outproj_rows(mis):
                """Out-projection + RS for global 128-row chunks mis.
                ysb casts split between DVE and the Scalar copy queue."""
                for m in mis:
                    q = next(qq for qq, (r0c, r) in enumerate(CHUNKS)
                             if r0c <= m * 128 < r0c + r)
                    r0c, rc = CHUNKS[q]
                    for nb in range(4):
                        yp = psp.tile([128, 512], F32, tag="psA", bufs=2,
                                      name=f"yp{m}_{nb}")
                        for h in range(HLOC):
                            nc.tensor.matmul(
                                yp, ot[h][:, m*128:(m+1)*128],
                                wo_t[h*4 + nb],
                                start=(h == 0), stop=(h == HLOC - 1))
                        ysb = ysp.tile([128, 512], BF16, tag="ysb",
                                       name=f"ysb{m}_{nb}")
                        if nb % 2 == 0:
                            nc.vector.tensor_copy(ysb, yp)
                        else:
                            nc.scalar.copy(ysb, yp)
                        r0 = m * 128 - r0c
                        nc.sync.dma_start(
                            ypart[q].ap()[r0:r0+128,
                                          nb*512:(nb+1)*512], ysb)
                    if (m + 1) * 128 == r0c + rc:
                        rs = nc.gpsimd.collective_compute(
                            "ReduceScatter", AOP.add,
                            replica_groups=GROUPS,
                            ins=[ypart[q].ap()], outs=[yred[q].ap()])
                        rs_insts.append(rs)

            # ---------------- pipelined attention ----------------------
            # Step u=(i0,w,h). Emission per steady step:
            #   tree(u); S(u+1) first half; O(u)[0:8]; S(u+1) second
            #   half; O(u)[8:16]; [qproj filler / outproj+RS]
            seq = [(n * 512, 512, h) for n in range(3) for h in range(HLOC)]
            seq += [(1536 + s * 256, 256, h) for s in range(2)
                    for h in range(HLOC)]

            # prologue: S(0,0) fully, then K pass B.
            pts_u = attn_S(0, 512, 0, range(8))
            kproj_pass([2, 3])
            # wo as 16 [128,512] tiles (h*4+nb), reusing wv's tile slots
            wo_t = []
            for h in range(HLOC):
                for nb in range(4):
                    t = wap.tile([128, HDL], BF16, name=f"wo{h}_{nb}",
                                 tag="wA", bufs=NK)
                    nc.sync.dma_start(
                        t, wo[h*128:(h+1)*128, nb*512:(nb+1)*512])
                    wo_t.append(t)

            for idx, (i0, w, h) in enumerate(seq):
                # current step u has pts_u ready (exp'd); emit its
                # reduction + O, interleaved with S of the next step.
                cs = attn_tree(i0, w, h, pts_u)
                if idx + 1 < len(seq):
                    i2, w2, h2 = seq[idx + 1]
                    t2 = 16 * w2 // 1024
                    pts_v = attn_S(i2, w2, h2, range(t2 // 2))
                op = psp.tile([128, 512], F32, tag="psO", bufs=2,
                              name=f"op{i0}_{h}")
                attn_O(i0, w, h, pts_u, op, cs, range(0, 8))
                if idx + 1 < len(seq):
                    pts_v += attn_S(i2, w2, h2, range(t2 // 2, t2))
                attn_O(i0, w, h, pts_u, op, cs, range(8, 16))
                if idx + 1 < len(seq):
                    pts_u = pts_v
                # splice projections / outproj into the PE stream
                if w == 512 and h == 1 and i0 // 512 + 1 < NI:
                    qproj_block(i0 // 512 + 1)
                if h == HLOC - 1:
                    outproj_rows(range(i0 // 128, (i0 + w) // 128))

            from concourse.bass import _add_dep_helper
            yo = 0
            for q, (_, r) in enumerate(CHUNKS):
                ydma = nc.gpsimd.dma_start(y[yo:yo + r // 4, :],
                                           yred[q].ap())
                yo += r // 4
                _add_dep_helper(
                    ydma.ins, rs_insts[-1].ins, sync=False,
                    reason="keep final y DMAs after all RS triggers")

    nc.compile()
    return nc


def get_program():
    if "nc" not in _CACHE:
        _CACHE["nc"] = _build()
    return _CACHE["nc"]


def make_in_maps(x_q, x_k, x_v, Wq, bq, Wk, bk, Wv, bv, Wo, bo):
    f = np.float32
    b16 = ml_dtypes.bfloat16
    x_q = np.asarray(x_q, f)
    x_k = np.asarray(x_k, f)
    x_v = np.asarray(x_v, f)
    Wq = np.asarray(Wq, f)
    Wk = np.asarray(Wk, f)
    Wv = np.asarray(Wv, f)
    Wo = np.asarray(Wo, f)
    bq = np.asarray(bq, f)
    bk = np.asarray(bk, f)
    bv = np.asarray(bv, f)
    xts = {}
    for b in range(B):
        xts[b] = (np.ascontiguousarray(x_q[b].T).astype(b16),
                  np.ascontiguousarray(x_k[b].T).astype(b16),
                  np.ascontiguousarray(x_v[b].T).astype(b16))
    in_maps = []
    for c in range(8):
        b, g = divmod(c, 4)
        cs = g * HDL
        sl = slice(cs, cs + HDL)
        in_maps.append({
            "xqt": xts[b][0], "xkt": xts[b][1], "xvt": xts[b][2],
            "wq": np.ascontiguousarray(Wq[:, sl]).astype(b16),
            "wk": np.ascontiguousarray(Wk[:, sl]).astype(b16),
            "wv": np.ascontiguousarray(Wv[:, sl]).astype(b16),
            "wo": np.ascontiguousarray(Wo[sl, :]).astype(b16),
            "bq2": np.ascontiguousarray(bq[sl].reshape(HLOC, 128, 1)),
            "bk2": np.ascontiguousarray(bk[sl].reshape(HLOC, 128, 1)),
            "bv2": np.ascontiguousarray(bv[sl].reshape(1, HDL)),
        })
    return in_maps


def assemble(results, bo):
    out = np.empty((B, L, D), np.float32)
    bo = np.asarray(bo, np.float32)
    for c in range(8):
        b, g = divmod(c, 4)
        yc = np.asarray(results[c]["y"], np.float32)
        yo = 0
        for r0c, r in CHUNKS:
            rr = r // 4
            r0 = r0c + g * rr
            out[b, r0:r0+rr, :] = yc[yo:yo+rr, :] + bo
            yo += rr
    return out


def kernel(**inputs) -> np.ndarray:
    from concourse.bass_utils import run_bass_kernel_spmd
    nc = get_program()
    in_maps = make_in_maps(**inputs)
    res = run_bass_kernel_spmd(nc, in_maps, list(range(8)))
    return assemble(res.results, inputs["bo"])
